# revision 12
# baseline (speedup 1.0000x reference)
"""Trainium2 Bass kernel for nn_DSRLossStateless (DSR loss, stateless).

loss = -sum_t(D_t)/B where D_t comes from an eta-EMA pair (A,B) over
portfolio returns R_t = sum_a w[t,a]*nr[t,a].

v4 strategy (8 cores, batch-sharded, interleaved layout):
  - Host: fp16 inputs; core m owns 250k rows + 2000 preceding rows
    (history; synthetic EPS-seed rows for core 0). Rows are interleaved
    so SBUF partition p holds times t === p (mod 125): tile [125, 2016]
    with t = col*125 + p. Per-partition DMA stays contiguous.
  - Persistent full-width input tiles; DMA sliced by column ranges over
    3 queues (sync/scalar HW DGE + one gpsimd SW-DGE share). Compute
    groups pipeline against the slices via region-level deps.
  - Per 504-col group: in-place fp16 product (DVE 2x) -> pairwise
    tree-sum to R (into dead rt space) -> R2=Square(R) on ACT -> PE
    matmuls with triangular geometric L',U' compute the 125-tap shifted
    FIR into PSUM (f32) -> DVE scans with decay c^125 give exact
    A_prev/B_prev (no carries, no tail fixup) -> D-chain:
    gp: g1=R2+B, g3=B*R, t1=A*g1, vv=B-A^2; ACT: a2=A^2, ln=Ln(vv),
    rc=Exp(-1.5*ln) (= var^-1.5); DVE: negn'=(2*g3-t1), qs += negn'*rc.
    Host applies loss = -0.5*eta*total/B (sign+half folded out).
  - Final: per-partition sums -> partition gather -> scalar out.
"""

import sys

sys.path.insert(0, "/opt/trn_rl_repo")

import numpy as np

import concourse.bass as bass
import concourse.bacc as bacc
import concourse.tile as tile
from concourse import mybir
from concourse.bass_utils import run_bass_kernel_spmd
from contextlib import ExitStack

F32 = mybir.dt.float32
F16 = mybir.dt.float16
NF32 = np.float32
NF16 = np.float16

N_CORES = 8
NA = 16
KP = 125                # partitions (time interleave stride)
PRE_COLS = 16
NC_COLS = 2000
COLS = NC_COLS + PRE_COLS   # 2016
OWN = KP * NC_COLS      # 250000
PRE = KP * PRE_COLS     # 2000
B_TOTAL = N_CORES * OWN
ETA = 0.01
EPS = 1e-8
CDEC = NF32(1.0 - ETA)
C125 = NF32(float(CDEC) ** KP)

# DMA column slices (per tensor) and compute groups (<=504 for one matmul;
# small last group keeps the drain short)
DMA_SLICES = [(0, 504), (504, 1008), (1008, 1512), (1512, 2016)]
GROUPS = [(0, 504), (504, 1008), (1008, 1512), (1512, 1848), (1848, 2016)]

AL = mybir.AluOpType
AF = mybir.ActivationFunctionType
AX = mybir.AxisListType

_PROGRAM = None


def _fir_matrices():
    """lhsT-layout [k, m] FIR matrices with eta folded in.

    A_prev[m, n] = c125*A_prev[m, n-1] + sum_k L[k,m] R[k,n] + U[k,m] R[k,n-1]
    """
    k = np.arange(KP)
    m = np.arange(KP)
    c = float(CDEC)
    Lm = np.where(k[:, None] <= m[None, :] - 1,
                  c ** (m[None, :] - 1 - k[:, None]), 0.0)
    Um = np.where(k[:, None] >= m[None, :],
                  c ** (KP + m[None, :] - 1 - k[:, None]), 0.0)
    return (ETA * Lm).astype(NF16), (ETA * Um).astype(NF16)


def _build_program():
    nc = bacc.Bacc("TRN2", target_bir_lowering=False, debug=False)

    # 126-row transfers spread across 14 DMA engines; 125-row collapse to 5.
    w_ap = nc.dram_tensor("w", [KP + 1, COLS * NA], F16, kind="ExternalInput").ap()
    nr_ap = nc.dram_tensor("nr", [KP + 1, COLS * NA], F16, kind="ExternalInput").ap()
    out_ap = nc.dram_tensor("out", [1, 1], F32, kind="ExternalOutput").ap()

    Lnp, Unp = _fir_matrices()
    L_dram = nc.inline_tensor(np.ascontiguousarray(Lnp), name="Lfir")
    U_dram = nc.inline_tensor(np.ascontiguousarray(Unp), name="Ufir")

    with tile.TileContext(nc) as tc, ExitStack() as ctx:
        pers = ctx.enter_context(tc.tile_pool(name="pers", bufs=1))
        dchp = ctx.enter_context(tc.tile_pool(name="dch", bufs=2))
        psump = ctx.enter_context(tc.psum_pool(name="ps", bufs=2))

        wt = pers.tile([KP + 1, COLS * NA], F16, tag="wt")
        rt = pers.tile([KP + 1, COLS * NA], F16, tag="rt")
        Lt = pers.tile([KP, KP], F16, tag="Lt")
        Ut = pers.tile([KP, KP], F16, tag="Ut")
        R = pers.tile([KP, 1 + COLS], F16, tag="R")
        R2 = pers.tile([KP, 1 + COLS], F16, tag="R2")
        Aprev = pers.tile([KP, COLS], F32, tag="Aprev")
        Bprev = pers.tile([KP, COLS], F32, tag="Bprev")
        CD = pers.tile([KP, 504], F32, tag="CD")
        qs = pers.tile([KP, len(GROUPS)], F32, tag="qs")
        qp = pers.tile([KP, 1], F32, tag="qp")
        qrow = pers.tile([1, KP], F32, tag="qrow")
        qtot = pers.tile([1, 1], F32, tag="qtot")

        nc.sync.dma_start(Lt[:], L_dram.ap())
        nc.sync.dma_start(Ut[:], U_dram.ap())
        nc.vector.memset(CD[:, :], float(C125))
        nc.vector.memset(R[:, 0:1], 0.0)
        nc.vector.memset(R2[:, 0:1], 0.0)
        # pin ACT tables before the pipeline
        nc.vector.memset(qtot[0:1, 0:1], 1.0)
        nc.scalar.activation(qtot[0:1, 0:1], qtot[0:1, 0:1], AF.Square)
        nc.scalar.activation(qtot[0:1, 0:1], qtot[0:1, 0:1], AF.Ln)
        nc.scalar.activation(qtot[0:1, 0:1], qtot[0:1, 0:1], AF.Exp)

        # input slices: w -> sync,sync,scalar,gp ; nr -> scalar,scalar,sync,gp
        wq = [nc.sync, nc.sync, nc.scalar, nc.gpsimd]
        rq = [nc.scalar, nc.scalar, nc.sync, nc.gpsimd]
        for i, (a, b) in enumerate(DMA_SLICES):
            wq[i].dma_start(wt[:, a * NA:b * NA], w_ap[:, a * NA:b * NA])
            rq[i].dma_start(rt[:, a * NA:b * NA], nr_ap[:, a * NA:b * NA])

        for g, (gs, ge) in enumerate(GROUPS):
            tcw = ge - gs
            rs = slice(1 + gs, 1 + ge)   # R/R2 (col 0 is the zero pad)
            sh = slice(gs, ge)           # shifted R/R2
            cs = slice(gs, ge)           # Aprev/Bprev/D-chain
            es, ee = gs * NA, ge * NA    # input element range

            # product (in-place into wt) + tree rowsum into dead rt space
            nc.vector.tensor_mul(wt[0:KP, es:ee], wt[0:KP, es:ee], rt[0:KP, es:ee])
            p3 = wt[0:KP, es:ee].rearrange("p (t a) -> p t a", a=16)
            u1 = rt[0:KP, es:es + tcw * 8].rearrange("p (t a) -> p t a", a=8)
            nc.vector.tensor_add(u1[:, :, :], p3[:, :, 0:8], p3[:, :, 8:16])
            nc.vector.tensor_add(u1[:, :, 0:4], u1[:, :, 0:4], u1[:, :, 4:8])
            nc.vector.tensor_add(u1[:, :, 0:2], u1[:, :, 0:2], u1[:, :, 2:4])
            nc.vector.tensor_add(R[:, rs], u1[:, :, 0], u1[:, :, 1])
            nc.scalar.activation(R2[:, rs], R[:, rs], AF.Square)

            # FIR matmuls into PSUM (f32 accumulate)
            YA = psump.tile([KP, 504], F32, tag="YA")
            YB = psump.tile([KP, 504], F32, tag="YB")
            nc.tensor.matmul(YA[:, 0:tcw], lhsT=Lt[:], rhs=R[:, rs],
                             start=True, stop=False)
            nc.tensor.matmul(YB[:, 0:tcw], lhsT=Lt[:], rhs=R2[:, rs],
                             start=True, stop=False)
            nc.tensor.matmul(YA[:, 0:tcw], lhsT=Ut[:], rhs=R[:, sh],
                             start=False, stop=True)
            nc.tensor.matmul(YB[:, 0:tcw], lhsT=Ut[:], rhs=R2[:, sh],
                             start=False, stop=True)

            # exact A_prev/B_prev via c^125-decay scans along columns
            initA = 0.0 if g == 0 else Aprev[:, gs - 1:gs]
            initB = 0.0 if g == 0 else Bprev[:, gs - 1:gs]
            nc.vector.tensor_tensor_scan(
                out=Aprev[:, cs], data0=CD[:, 0:tcw], data1=YA[:, 0:tcw],
                initial=initA, op0=AL.mult, op1=AL.add,
            )
            nc.vector.tensor_tensor_scan(
                out=Bprev[:, cs], data0=CD[:, 0:tcw], data1=YB[:, 0:tcw],
                initial=initB, op0=AL.mult, op1=AL.add,
            )

            # D-chain: qs += (2*g3 - t1) * var^-1.5  (negated q; host flips)
            g1 = dchp.tile([KP, 504], F32, tag="g1")
            g3 = dchp.tile([KP, 504], F32, tag="g3")
            t1 = dchp.tile([KP, 504], F32, tag="t1")
            a2 = dchp.tile([KP, 504], F32, tag="a2")
            vv = dchp.tile([KP, 504], F32, tag="vv")
            lnv = dchp.tile([KP, 504], F32, tag="lnv")
            rc = dchp.tile([KP, 504], F32, tag="rc")
            nc.gpsimd.tensor_add(g1[:, 0:tcw], R2[:, rs], Bprev[:, cs])
            nc.gpsimd.tensor_mul(g3[:, 0:tcw], Bprev[:, cs], R[:, rs])
            nc.gpsimd.tensor_mul(t1[:, 0:tcw], Aprev[:, cs], g1[:, 0:tcw])
            nc.scalar.activation(a2[:, 0:tcw], Aprev[:, cs], AF.Square)
            nc.gpsimd.tensor_sub(vv[:, 0:tcw], Bprev[:, cs], a2[:, 0:tcw])
            # var^-1.5 = exp(-1.5*ln(var)); vv==0 only on excluded prepend cols
            nc.scalar.activation(lnv[:, 0:tcw], vv[:, 0:tcw], AF.Ln)
            nc.scalar.activation(rc[:, 0:tcw], lnv[:, 0:tcw], AF.Exp, scale=-1.5)
            nc.vector.scalar_tensor_tensor(
                out=t1[:, 0:tcw], in0=g3[:, 0:tcw], scalar=2.0, in1=t1[:, 0:tcw],
                op0=AL.mult, op1=AL.subtract,
            )
            qa = slice(PRE_COLS, tcw) if g == 0 else slice(0, tcw)
            nc.vector.scalar_tensor_tensor(
                out=rc[:, qa], in0=t1[:, qa], scalar=1.0, in1=rc[:, qa],
                op0=AL.mult, op1=AL.mult, accum_out=qs[:, g:g + 1],
            )

        nc.vector.reduce_sum(qp[:, 0:1], qs[:, :], axis=AX.X)
        nc.sync.dma_start(qrow[0:1, 0:KP], qp[0:KP, 0:1])
        nc.vector.reduce_sum(qtot[0:1, 0:1], qrow[0:1, 0:KP], axis=AX.X)
        nc.sync.dma_start(out_ap[0:1, 0:1], qtot[0:1, 0:1])

    nc.compile()
    return nc


def _get_program():
    global _PROGRAM
    if _PROGRAM is None:
        _PROGRAM = _build_program()
    return _PROGRAM


def _core0_prepend():
    """2000 synthetic history rows encoding the global init (A,B)=(0,EPS)."""
    w = np.zeros((PRE, NA), NF32)
    nr = np.zeros((PRE, NA), NF32)
    c = CDEC
    r1 = NF32(np.sqrt(EPS / (ETA * (float(c) + float(c) ** 2))))
    r2 = NF32(-(c * r1))
    w[PRE - 2, 0] = NF32(1.0)
    nr[PRE - 2, 0] = r1
    w[PRE - 1, 0] = NF32(1.0)
    nr[PRE - 1, 0] = r2
    return w, nr


def _interleave16(arr):
    # [COLS*KP, NA] f32 -> [KP+1, COLS*NA] fp16, t = col*KP + p; row KP pad
    out = np.zeros((KP + 1, COLS * NA), NF16)
    out[:KP] = np.ascontiguousarray(
        arr.reshape(COLS, KP, NA).transpose(1, 0, 2).astype(NF16)
    ).reshape(KP, COLS * NA)
    return out


def _make_in_maps(weights, nr):
    weights = np.asarray(weights, dtype=NF32)
    nr = np.asarray(nr, dtype=NF32)
    pre_w, pre_nr = _core0_prepend()
    in_maps = []
    for m in range(N_CORES):
        s = m * OWN
        if m == 0:
            wm = np.concatenate([pre_w, weights[:OWN]])
            rm = np.concatenate([pre_nr, nr[:OWN]])
        else:
            wm = weights[s - PRE:s + OWN]
            rm = nr[s - PRE:s + OWN]
        in_maps.append({"w": _interleave16(wm), "nr": _interleave16(rm)})
    return in_maps


def _run(in_maps, **kwargs):
    nc = _get_program()
    return run_bass_kernel_spmd(nc, in_maps, core_ids=list(range(N_CORES)), **kwargs)


def kernel(weights, next_returns):
    in_maps = _make_in_maps(weights, next_returns)
    res = _run(in_maps)
    total = np.sum(
        np.array([res.results[m]["out"][0, 0] for m in range(N_CORES)], NF32),
        dtype=NF32,
    )
    # device accumulated (2*g3 - t1)*rc = -2*negn*rc; fold 0.5 and sign here
    return NF32(NF32(-0.5) * NF32(ETA) * total / NF32(B_TOTAL))


# revision 13
# speedup vs baseline: 1.1492x; 1.1492x over previous
"""Trainium2 Bass kernel for nn_DSRLossStateless (DSR loss, stateless).

loss = -sum_t(D_t)/B where D_t comes from an eta-EMA pair (A,B) over
portfolio returns R_t = sum_a w[t,a]*nr[t,a].

v5 strategy (8 cores, batch-sharded, interleaved layout):
  - Host: fp16 inputs with w pre-scaled by 2 (so the device stores
    Rt=2R and negn = B*Rt - A*(R^2+B) needs no scalar op); core m owns
    250k rows + 2000 preceding rows (history; synthetic EPS-seed rows
    for core 0). Rows are interleaved so SBUF partition p holds times
    t === p (mod 125): tile [125, 2016] with t = col*125 + p.
  - Per column-group ([168,504,504,504,336], own double-buffered load
    tiles, w->sync / nr->scalar HW DMA queues, ~350GB/s):
    in-place fp16 product (DVE 2x) -> pairwise tree-sum to Rt (into
    dead rt space) -> R2 = Square(0.5*Rt) on ACT -> PE matmuls with
    triangular geometric matrices (A-side halved for Rt) compute the
    125-tap shifted FIR into PSUM (f32) -> DVE scans with decay c^125
    give exact A_prev/B_prev (no carries, no tail fixup) -> D-chain:
    gp: g1=R2+B, g3=B*Rt, t1=A*g1, vv=B-a2, negn=g3-t1;
    ACT: a2=A^2, ln=Ln(vv), rc=Exp(-1.5*ln) (= var^-1.5);
    DVE: qs += negn*rc.  Host: loss = -0.5*eta*total/B.
  - Final: per-partition sums -> partition gather -> scalar out.
"""

import sys

sys.path.insert(0, "/opt/trn_rl_repo")

import numpy as np

import concourse.bass as bass
import concourse.bacc as bacc
import concourse.tile as tile
from concourse import mybir
from concourse.bass_utils import run_bass_kernel_spmd
from contextlib import ExitStack

F32 = mybir.dt.float32
F16 = mybir.dt.float16
NF32 = np.float32
NF16 = np.float16

N_CORES = 8
NA = 16
KP = 125                # partitions (time interleave stride)
PRE_COLS = 16
NC_COLS = 2000
COLS = NC_COLS + PRE_COLS   # 2016
OWN = KP * NC_COLS      # 250000
PRE = KP * PRE_COLS     # 2000
B_TOTAL = N_CORES * OWN
ETA = 0.01
EPS = 1e-8
CDEC = NF32(1.0 - ETA)
C125 = NF32(float(CDEC) ** KP)

# compute groups (<=504 cols for one matmul); small first group -> early
# pipeline start, small last group -> short drain
GROUPS = [(0, 168), (168, 672), (672, 1176), (1176, 1680), (1680, 2016)]

AL = mybir.AluOpType
AF = mybir.ActivationFunctionType
AX = mybir.AxisListType

_PROGRAM = None


def _fir_matrices():
    """lhsT-layout [k, m] FIR matrices.

    A_prev[m,n] = c125*A_prev[m,n-1] + sum_k La[k,m] Rt[k,n] + Ua[k,m] Rt[k,n-1]
    B_prev[m,n] = c125*B_prev[m,n-1] + sum_k Lb[k,m] R2[k,n] + Ub[k,m] R2[k,n-1]
    with Rt = 2R, so the A-side matrices carry eta/2 and the B-side eta.
    """
    k = np.arange(KP)
    m = np.arange(KP)
    c = float(CDEC)
    Lm = np.where(k[:, None] <= m[None, :] - 1,
                  c ** (m[None, :] - 1 - k[:, None]), 0.0)
    Um = np.where(k[:, None] >= m[None, :],
                  c ** (KP + m[None, :] - 1 - k[:, None]), 0.0)
    return ((0.5 * ETA * Lm).astype(NF16), (ETA * Lm).astype(NF16),
            (0.5 * ETA * Um).astype(NF16), (ETA * Um).astype(NF16))


def _build_program():
    nc = bacc.Bacc("TRN2", target_bir_lowering=False, debug=False)

    # 126-row transfers spread across 14 DMA engines; 125-row collapse to 5.
    w_ap = nc.dram_tensor("w", [KP + 1, COLS * NA], F16, kind="ExternalInput").ap()
    nr_ap = nc.dram_tensor("nr", [KP + 1, COLS * NA], F16, kind="ExternalInput").ap()
    out_ap = nc.dram_tensor("out", [1, 1], F32, kind="ExternalOutput").ap()

    La_np, Lb_np, Ua_np, Ub_np = _fir_matrices()
    La_d = nc.inline_tensor(np.ascontiguousarray(La_np), name="Lafir")
    Lb_d = nc.inline_tensor(np.ascontiguousarray(Lb_np), name="Lbfir")
    Ua_d = nc.inline_tensor(np.ascontiguousarray(Ua_np), name="Uafir")
    Ub_d = nc.inline_tensor(np.ascontiguousarray(Ub_np), name="Ubfir")

    with tile.TileContext(nc) as tc, ExitStack() as ctx:
        pers = ctx.enter_context(tc.tile_pool(name="pers", bufs=1))
        loadp = ctx.enter_context(tc.tile_pool(name="load", bufs=3))
        dchp = ctx.enter_context(tc.tile_pool(name="dch", bufs=2))
        psump = ctx.enter_context(tc.psum_pool(name="ps", bufs=2))

        La = pers.tile([KP, KP], F16, tag="La")
        Lb = pers.tile([KP, KP], F16, tag="Lb")
        Ua = pers.tile([KP, KP], F16, tag="Ua")
        Ub = pers.tile([KP, KP], F16, tag="Ub")
        R = pers.tile([KP, 1 + COLS], F16, tag="R")
        R2 = pers.tile([KP, 1 + COLS], F16, tag="R2")
        Aprev = pers.tile([KP, COLS], F32, tag="Aprev")
        Bprev = pers.tile([KP, COLS], F32, tag="Bprev")
        CD = pers.tile([KP, 504], F32, tag="CD")
        qs = pers.tile([KP, len(GROUPS)], F32, tag="qs")
        qp = pers.tile([KP, 1], F32, tag="qp")
        qrow = pers.tile([1, KP], F32, tag="qrow")
        qtot = pers.tile([1, 1], F32, tag="qtot")

        nc.sync.dma_start(La[:], La_d.ap())
        nc.sync.dma_start(Lb[:], Lb_d.ap())
        nc.sync.dma_start(Ua[:], Ua_d.ap())
        nc.sync.dma_start(Ub[:], Ub_d.ap())
        nc.vector.memset(CD[:, :], float(C125))
        nc.vector.memset(R[:, 0:1], 0.0)
        nc.vector.memset(R2[:, 0:1], 0.0)
        # pin ACT tables before the pipeline
        nc.vector.memset(qtot[0:1, 0:1], 1.0)
        nc.scalar.activation(qtot[0:1, 0:1], qtot[0:1, 0:1], AF.Square)
        nc.scalar.activation(qtot[0:1, 0:1], qtot[0:1, 0:1], AF.Ln)
        nc.scalar.activation(qtot[0:1, 0:1], qtot[0:1, 0:1], AF.Exp)

        for g, (gs, ge) in enumerate(GROUPS):
            tcw = ge - gs
            rs = slice(1 + gs, 1 + ge)   # R/R2 (col 0 is the zero pad)
            sh = slice(gs, ge)           # shifted R/R2
            cs = slice(gs, ge)           # Aprev/Bprev/D-chain

            wt = loadp.tile([KP + 1, 504 * NA], F16, tag="wt")
            rt = loadp.tile([KP + 1, 504 * NA], F16, tag="rt")
            ew = tcw * NA
            nc.sync.dma_start(wt[:, 0:ew], w_ap[:, gs * NA:ge * NA])
            nc.scalar.dma_start(rt[:, 0:ew], nr_ap[:, gs * NA:ge * NA])

            # product (in-place into wt) + tree rowsum into dead rt space
            nc.vector.tensor_mul(wt[0:KP, 0:ew], wt[0:KP, 0:ew], rt[0:KP, 0:ew])
            p3 = wt[0:KP, 0:ew].rearrange("p (t a) -> p t a", a=16)
            u1 = rt[0:KP, 0:tcw * 8].rearrange("p (t a) -> p t a", a=8)
            nc.vector.tensor_add(u1[:, :, :], p3[:, :, 0:8], p3[:, :, 8:16])
            nc.vector.tensor_add(u1[:, :, 0:4], u1[:, :, 0:4], u1[:, :, 4:8])
            nc.vector.tensor_add(u1[:, :, 0:2], u1[:, :, 0:2], u1[:, :, 2:4])
            nc.vector.tensor_add(R[:, rs], u1[:, :, 0], u1[:, :, 1])
            # R tile holds Rt=2R; R2 = Square(0.5*Rt) is the exact R^2
            nc.scalar.activation(R2[:, rs], R[:, rs], AF.Square, scale=0.5)

            # FIR matmuls into PSUM (f32 accumulate)
            YA = psump.tile([KP, 504], F32, tag="YA")
            YB = psump.tile([KP, 504], F32, tag="YB")
            nc.tensor.matmul(YA[:, 0:tcw], lhsT=La[:], rhs=R[:, rs],
                             start=True, stop=False)
            nc.tensor.matmul(YB[:, 0:tcw], lhsT=Lb[:], rhs=R2[:, rs],
                             start=True, stop=False)
            nc.tensor.matmul(YA[:, 0:tcw], lhsT=Ua[:], rhs=R[:, sh],
                             start=False, stop=True)
            nc.tensor.matmul(YB[:, 0:tcw], lhsT=Ub[:], rhs=R2[:, sh],
                             start=False, stop=True)

            # exact A_prev/B_prev via c^125-decay scans along columns
            initA = 0.0 if g == 0 else Aprev[:, gs - 1:gs]
            initB = 0.0 if g == 0 else Bprev[:, gs - 1:gs]
            nc.vector.tensor_tensor_scan(
                out=Aprev[:, cs], data0=CD[:, 0:tcw], data1=YA[:, 0:tcw],
                initial=initA, op0=AL.mult, op1=AL.add,
            )
            nc.vector.tensor_tensor_scan(
                out=Bprev[:, cs], data0=CD[:, 0:tcw], data1=YB[:, 0:tcw],
                initial=initB, op0=AL.mult, op1=AL.add,
            )

            # D-chain: qs += (B*Rt - A*(R2+B)) * var^-1.5   (= 2*D/eta)
            g1 = dchp.tile([KP, 504], F32, tag="g1")
            g3 = dchp.tile([KP, 504], F32, tag="g3")
            t1 = dchp.tile([KP, 504], F32, tag="t1")
            a2 = dchp.tile([KP, 504], F32, tag="a2")
            vv = dchp.tile([KP, 504], F32, tag="vv")
            lnv = dchp.tile([KP, 504], F32, tag="lnv")
            rc = dchp.tile([KP, 504], F32, tag="rc")
            nc.gpsimd.tensor_add(g1[:, 0:tcw], R2[:, rs], Bprev[:, cs])
            nc.gpsimd.tensor_mul(g3[:, 0:tcw], Bprev[:, cs], R[:, rs])
            nc.gpsimd.tensor_mul(t1[:, 0:tcw], Aprev[:, cs], g1[:, 0:tcw])
            nc.scalar.activation(a2[:, 0:tcw], Aprev[:, cs], AF.Square)
            nc.gpsimd.tensor_sub(vv[:, 0:tcw], Bprev[:, cs], a2[:, 0:tcw])
            nc.gpsimd.tensor_sub(g3[:, 0:tcw], g3[:, 0:tcw], t1[:, 0:tcw])
            # var^-1.5 = exp(-1.5*ln(var)); vv==0 only on excluded prepend cols
            nc.scalar.activation(lnv[:, 0:tcw], vv[:, 0:tcw], AF.Ln)
            nc.scalar.activation(rc[:, 0:tcw], lnv[:, 0:tcw], AF.Exp, scale=-1.5)
            qa = slice(PRE_COLS, tcw) if g == 0 else slice(0, tcw)
            nc.vector.scalar_tensor_tensor(
                out=rc[:, qa], in0=g3[:, qa], scalar=1.0, in1=rc[:, qa],
                op0=AL.mult, op1=AL.mult, accum_out=qs[:, g:g + 1],
            )

        nc.vector.reduce_sum(qp[:, 0:1], qs[:, :], axis=AX.X)
        nc.sync.dma_start(qrow[0:1, 0:KP], qp[0:KP, 0:1])
        nc.vector.reduce_sum(qtot[0:1, 0:1], qrow[0:1, 0:KP], axis=AX.X)
        nc.sync.dma_start(out_ap[0:1, 0:1], qtot[0:1, 0:1])

    nc.compile()
    return nc


def _get_program():
    global _PROGRAM
    if _PROGRAM is None:
        _PROGRAM = _build_program()
    return _PROGRAM


def _core0_prepend():
    """2000 synthetic history rows encoding the global init (A,B)=(0,EPS)."""
    w = np.zeros((PRE, NA), NF32)
    nr = np.zeros((PRE, NA), NF32)
    c = CDEC
    r1 = NF32(np.sqrt(EPS / (ETA * (float(c) + float(c) ** 2))))
    r2 = NF32(-(c * r1))
    w[PRE - 2, 0] = NF32(1.0)
    nr[PRE - 2, 0] = r1
    w[PRE - 1, 0] = NF32(1.0)
    nr[PRE - 1, 0] = r2
    return w, nr


def _interleave16(arr, scale=1.0):
    # [COLS*KP, NA] f32 -> [KP+1, COLS*NA] fp16, t = col*KP + p; row KP pad
    out = np.zeros((KP + 1, COLS * NA), NF16)
    out[:KP] = np.ascontiguousarray(
        (arr.reshape(COLS, KP, NA) * NF32(scale)).transpose(1, 0, 2).astype(NF16)
    ).reshape(KP, COLS * NA)
    return out


def _make_in_maps(weights, nr):
    weights = np.asarray(weights, dtype=NF32)
    nr = np.asarray(nr, dtype=NF32)
    pre_w, pre_nr = _core0_prepend()
    in_maps = []
    for m in range(N_CORES):
        s = m * OWN
        if m == 0:
            wm = np.concatenate([pre_w, weights[:OWN]])
            rm = np.concatenate([pre_nr, nr[:OWN]])
        else:
            wm = weights[s - PRE:s + OWN]
            rm = nr[s - PRE:s + OWN]
        in_maps.append({"w": _interleave16(wm, 2.0), "nr": _interleave16(rm)})
    return in_maps


def _run(in_maps, **kwargs):
    nc = _get_program()
    return run_bass_kernel_spmd(nc, in_maps, core_ids=list(range(N_CORES)), **kwargs)


def kernel(weights, next_returns):
    in_maps = _make_in_maps(weights, next_returns)
    res = _run(in_maps)
    total = np.sum(
        np.array([res.results[m]["out"][0, 0] for m in range(N_CORES)], NF32),
        dtype=NF32,
    )
    # device accumulated 2*D/eta per step; fold the 0.5 and the -mean here
    return NF32(NF32(-0.5) * NF32(ETA) * total / NF32(B_TOTAL))
